# revision 6
# baseline (speedup 1.0000x reference)
"""Trainium2 Bass kernel for nn_BasicBlockLogS (log-polar pooling block).

Math: the reference module (log_pooling -> conv1(stride 4,3) + center 1x1 conv
+ bias -> training-mode BatchNorm -> relu(out + x)) collapses exactly into a
9x9 conv whose taps are partitioned into 12 log-polar bins (taps in a bin share
one weight matrix, scaled 1/|bin|) plus a center 1x1 matrix.  b_center cancels
inside BatchNorm.  Each bin is 1-2 rectangular blocks of taps, so the conv is
computed as 21 accumulated matmuls over [C=256] per output tile, with rhs =
horizontal/vertical run-sum images of x (computed once on the Vector engine and
shared by all output channels).

Sharding: pure data parallel, batch 32 -> 4 per core across 8 cores.  BN batch
stats (per-channel mean / E[x^2]) are all-reduced across cores on-device.

The conv datapath (x frames, run images, weights) is bf16: DVE tensor_tensor
gets 2x mode, and bf16 weights use fast-weight-load so LDWEIGHTS hides under
the matmul stream.  PSUM accumulation, BN statistics, the residual x, and the
final output stay fp32.
"""

import os
import sys
import types
import numpy as np
from contextlib import ExitStack

for _p in ("/opt/trn_rl_repo",):
    if _p not in sys.path:
        sys.path.insert(0, _p)

import ml_dtypes
import concourse.bass as bass
import concourse.tile as tile
from concourse import bacc, mybir
from concourse.bass_utils import run_bass_kernel_spmd

F32 = mybir.dt.float32
BF16 = mybir.dt.bfloat16

NCORES = 8
B, C, H, W = 32, 256, 28, 28
BLOC = B // NCORES            # 4 batch items per core
CB = 2                        # channel blocks of 128 (contraction)
MB = 2                        # output-channel blocks of 128
HHALF = 14                    # output rows per half-frame
FR = HHALF + 8                # padded rows per half-frame (22)
NT = HHALF * W                # N per matmul tile (392)
EPS = 1e-5

# log-polar bin sizes (taps per bin), bins k=0..11 (k = bh*3+bw order)
BIN_N = np.array([2, 1, 1, 2, 1, 1, 14, 11, 11, 14, 11, 11], np.float32)

# Segment table: (weight idx 0..12 [12=center], source, row offset, col offset)
# source: one of the run images, or 'xp' (padded x frame, col offset 4+dx).
# For a segment reading tensor G at vertical tap offset dy, local row = h'+ofs.
SEGS = [
    (9,  "v4L3", 1, 0),   # bin9  rows dy=-3..0,  cols [-4..-2]
    (9,  "L2",   0, 0),   # bin9  row  dy=-4,     cols [-4,-3]
    (10, "v2C3", 1, 0),   # bin10 rows dy=-3..-2, cols [-1..1]
    (10, "C5",   0, 0),   # bin10 row  dy=-4,     cols [-2..2]
    (11, "v3R3", 1, 0),   # bin11 rows dy=-3..-1, cols [2..4]
    (11, "R2",   0, 0),   # bin11 row  dy=-4,     cols [3,4]
    (6,  "v4R3", 4, 0),   # bin6  rows dy=0..3,   cols [2..4]
    (6,  "R2",   8, 0),   # bin6  row  dy=+4,     cols [3,4]
    (7,  "v2C3", 6, 0),   # bin7  rows dy=+2..+3, cols [-1..1]
    (7,  "C5",   8, 0),   # bin7  row  dy=+4,     cols [-2..2]
    (8,  "v3L3", 5, 0),   # bin8  rows dy=+1..+3, cols [-4..-2]
    (8,  "L2",   8, 0),   # bin8  row  dy=+4,     cols [-4,-3]
    (0,  "xp",   4, 1),   # bin0  (0,+1)
    (0,  "xp",   5, 1),   # bin0  (1,+1)
    (1,  "xp",   5, 0),   # bin1  (1,0)
    (2,  "xp",   5, -1),  # bin2  (1,-1)
    (3,  "xp",   3, -1),  # bin3  (-1,-1)
    (3,  "xp",   4, -1),  # bin3  (0,-1)
    (4,  "xp",   3, 0),   # bin4  (-1,0)
    (5,  "xp",   3, 1),   # bin5  (-1,1)
    (12, "xp",   4, 0),   # center 1x1
]


def _install_ntff_hook():
    """Register the axon NTFF profiling hook (absent antenv.axon_hooks shim)."""
    if "antenv.axon_hooks" in sys.modules:
        return
    mod = types.ModuleType("antenv.axon_hooks")
    mod._hook = None
    mod.set_axon_ntff_profile_hook = lambda h: setattr(mod, "_hook", h)
    mod.get_axon_ntff_profile_hook = lambda: mod._hook
    sys.modules["antenv.axon_hooks"] = mod
    try:
        from trn_agent_boot.trn_boot import _ntff_profile_via_ctypes
        mod.set_axon_ntff_profile_hook(
            _ntff_profile_via_ctypes("/opt/axon/libaxon_pjrt.so"))
    except Exception:
        pass


def build_program():
    nc = bacc.Bacc("TRN2", target_bir_lowering=False, debug=False,
                   num_devices=NCORES)

    x_in = nc.dram_tensor("x", [BLOC, C, H, W], F32, kind="ExternalInput").ap()
    xb_in = nc.dram_tensor("xb", [BLOC, C, H, W], BF16, kind="ExternalInput").ap()
    w1_in = nc.dram_tensor("w1t", [12, C, C], BF16, kind="ExternalInput").ap()
    wc_in = nc.dram_tensor("wct", [C, C], BF16, kind="ExternalInput").ap()
    g_in = nc.dram_tensor("gamma", [C], F32, kind="ExternalInput").ap()
    bt_in = nc.dram_tensor("beta", [C], F32, kind="ExternalInput").ap()
    out_d = nc.dram_tensor("out", [BLOC, C, H, W], F32, kind="ExternalOutput").ap()

    cc_in_d = nc.dram_tensor("cc_in", [128, 2 * MB], F32)
    cc_out_d = nc.dram_tensor("cc_out", [128, 2 * MB], F32, addr_space="Shared")

    # DRAM views with channels on partitions
    x_cbhw = x_in.rearrange("b c h w -> c b h w")
    xb_cbhw = xb_in.rearrange("b c h w -> c b h w")
    out_cbhw = out_d.rearrange("b c h w -> c b (h w)")

    with tile.TileContext(nc) as tc:
        with ExitStack() as ctx:
            persist = ctx.enter_context(tc.tile_pool(name="persist", bufs=1))
            stage = ctx.enter_context(tc.tile_pool(name="stage", bufs=2))
            trans = ctx.enter_context(tc.tile_pool(name="trans", bufs=7))
            psum = ctx.enter_context(tc.tile_pool(name="psum", bufs=4, space="PSUM"))
            small = ctx.enter_context(tc.tile_pool(name="small", bufs=1))

            # ---- persistent tiles ----
            w_all = persist.tile([128, CB, 13, C], BF16)     # lhsT: [c, p] per k
            gb = persist.tile([128, MB, 2], F32)             # gamma, beta
            out_sb = persist.tile([128, MB, BLOC, 2, NT], F32)
            x_res = persist.tile([128, MB, BLOC, 2, NT], F32)
            sb_stats = persist.tile([128, MB, BLOC * 2, 6], F32)
            eps_t = small.tile([128, 1], F32)
            nc.vector.memset(eps_t[:], EPS)

            # x frame slots (one per half): zero once; per-group DMA rewrites
            # only the interior, pads stay zero.
            xp_slots = [persist.tile([128, CB, FR, 36], BF16, name=f"xps{i}")
                        for i in range(2)]
            for i in range(2):
                nc.gpsimd.memset(xp_slots[i][:], 0.0)

            # ---- weights in ----
            for k in range(12):
                for cb in range(CB):
                    nc.sync.dma_start(
                        out=w_all[:, cb, k, :],
                        in_=w1_in[k, cb * 128:(cb + 1) * 128, :])
            for cb in range(CB):
                nc.sync.dma_start(
                    out=w_all[:, cb, 12, :],
                    in_=wc_in[cb * 128:(cb + 1) * 128, :])
            nc.sync.dma_start(out=gb[:, :, 0],
                              in_=g_in.rearrange("(cb c) -> c cb", c=128))
            nc.sync.dma_start(out=gb[:, :, 1],
                              in_=bt_in.rearrange("(cb c) -> c cb", c=128))

            # ---- main loop over (batch item, half) groups ----
            for b in range(BLOC):
                for half in range(2):
                    g = b * 2 + half
                    H0 = half * HHALF
                    # padded frame rows H0..H0+21; interior x rows:
                    xr0 = max(H0 - 4, 0)            # first x row
                    xr1 = min(H0 + FR - 4, H) - 1   # last x row (incl)
                    nxr = xr1 - xr0 + 1             # 18
                    l0 = xr0 + 4 - H0               # local row of first x row

                    xp = xp_slots[half]
                    for cb in range(CB):
                        nc.sync.dma_start(
                            out=xp[:, cb, l0:l0 + nxr, 4:32],
                            in_=xb_cbhw[cb * 128:(cb + 1) * 128, b,
                                        xr0:xr0 + nxr, :])
                        # fp32 x interior for the residual (rows of this half)
                        nc.sync.dma_start(
                            out=x_res[:, cb, b, half, :],
                            in_=x_cbhw[cb * 128:(cb + 1) * 128, b,
                                       H0:H0 + HHALF, :])

                    # ---- horizontal run sums (DVE) ----
                    def st(tag):
                        return stage.tile([128, CB, FR, W], BF16, name=tag, tag=tag)

                    def tr(tag):
                        return trans.tile([128, CB, FR, W], BF16, name=tag, tag="tmp")

                    L2 = st("L2")
                    nc.vector.tensor_add(L2[:], xp[:, :, :, 0:28], xp[:, :, :, 1:29])
                    L3 = tr("L3")
                    nc.vector.tensor_add(L3[:], L2[:], xp[:, :, :, 2:30])
                    C3 = tr("C3")
                    nc.vector.tensor_add(C3[:], xp[:, :, :, 3:31], xp[:, :, :, 4:32])
                    nc.vector.tensor_add(C3[:], C3[:], xp[:, :, :, 5:33])
                    C5 = st("C5")
                    nc.vector.tensor_add(C5[:], C3[:], xp[:, :, :, 2:30])
                    nc.vector.tensor_add(C5[:], C5[:], xp[:, :, :, 6:34])
                    R2 = st("R2")
                    nc.vector.tensor_add(R2[:], xp[:, :, :, 7:35], xp[:, :, :, 8:36])
                    R3 = tr("R3")
                    nc.vector.tensor_add(R3[:], R2[:], xp[:, :, :, 6:34])

                    # ---- vertical run sums ----
                    v2C3 = st("v2C3")
                    nc.vector.tensor_add(v2C3[:, :, 0:FR - 1, :],
                                         C3[:, :, 0:FR - 1, :], C3[:, :, 1:FR, :])
                    v2L3 = tr("v2L3")
                    nc.vector.tensor_add(v2L3[:, :, 0:FR - 1, :],
                                         L3[:, :, 0:FR - 1, :], L3[:, :, 1:FR, :])
                    v3L3 = st("v3L3")
                    nc.vector.tensor_add(v3L3[:, :, 0:FR - 2, :],
                                         v2L3[:, :, 0:FR - 2, :], L3[:, :, 2:FR, :])
                    v4L3 = st("v4L3")
                    nc.vector.tensor_add(v4L3[:, :, 0:FR - 3, :],
                                         v2L3[:, :, 0:FR - 3, :],
                                         v2L3[:, :, 2:FR - 1, :])
                    v2R3 = tr("v2R3")
                    nc.vector.tensor_add(v2R3[:, :, 0:FR - 1, :],
                                         R3[:, :, 0:FR - 1, :], R3[:, :, 1:FR, :])
                    v3R3 = st("v3R3")
                    nc.vector.tensor_add(v3R3[:, :, 0:FR - 2, :],
                                         v2R3[:, :, 0:FR - 2, :], R3[:, :, 2:FR, :])
                    v4R3 = st("v4R3")
                    nc.vector.tensor_add(v4R3[:, :, 0:FR - 3, :],
                                         v2R3[:, :, 0:FR - 3, :],
                                         v2R3[:, :, 2:FR - 1, :])

                    runs = {"L2": L2, "C5": C5, "R2": R2, "v2C3": v2C3,
                            "v3L3": v3L3, "v4L3": v4L3, "v3R3": v3R3,
                            "v4R3": v4R3, "xp": xp}

                    # ---- 21 segments x 2 cblk accumulated matmuls ----
                    for mb in range(MB):
                        ps = psum.tile([128, NT], F32, tag="ps")
                        n_mm = len(SEGS) * CB
                        si = 0
                        for (wi, src, ro, co) in SEGS:
                            tsrc = runs[src]
                            for cb in range(CB):
                                if src == "xp":
                                    rhs = tsrc[:, cb, ro:ro + HHALF,
                                               4 + co:4 + co + W]
                                else:
                                    rhs = tsrc[:, cb, ro:ro + HHALF, 0:W]
                                nc.tensor.matmul(
                                    ps[:],
                                    lhsT=w_all[:, cb, wi,
                                               mb * 128:(mb + 1) * 128],
                                    rhs=rhs,
                                    start=(si == 0), stop=(si == n_mm - 1))
                                si += 1
                        # per-tile batch-norm partial stats + copy out of PSUM
                        nc.vector.bn_stats(out=sb_stats[:, mb, g, :], in_=ps[:])
                        nc.scalar.copy(out=out_sb[:, mb, b, half, :], in_=ps[:])

            # ---- local stats -> (mean, E[x^2]) -> AllReduce ----
            mv = small.tile([128, MB, 2], F32)
            for mb in range(MB):
                nc.vector.bn_aggr(out=mv[:, mb, :], in_=sb_stats[:, mb, :, :])
            pack = small.tile([128, MB, 2], F32)
            nc.vector.tensor_copy(pack[:, :, 0:1], mv[:, :, 0:1])
            nc.vector.tensor_mul(pack[:, :, 1:2], mv[:, :, 0:1], mv[:, :, 0:1])
            nc.vector.tensor_add(pack[:, :, 1:2], pack[:, :, 1:2], mv[:, :, 1:2])
            nc.sync.dma_start(out=cc_in_d.ap(),
                              in_=pack[:].rearrange("p a b -> p (a b)"))
            nc.gpsimd.collective_compute(
                "AllReduce", mybir.AluOpType.add,
                replica_groups=[list(range(NCORES))],
                ins=[cc_in_d.ap()], outs=[cc_out_d.ap()])
            glob = small.tile([128, MB, 2], F32)
            nc.sync.dma_start(out=glob[:].rearrange("p a b -> p (a b)"),
                              in_=cc_out_d.ap())

            # global mean / var -> alpha, bias
            ge = small.tile([128, MB, 2], F32)
            nc.vector.tensor_scalar_mul(ge[:], glob[:], 1.0 / NCORES)
            var_g = small.tile([128, MB, 1], F32)
            nc.vector.tensor_mul(var_g[:], ge[:, :, 0:1], ge[:, :, 0:1])
            nc.vector.tensor_sub(var_g[:], ge[:, :, 1:2], var_g[:])
            alpha = small.tile([128, MB, 1], F32)
            nc.scalar.activation(out=alpha[:], in_=var_g[:],
                                 func=mybir.ActivationFunctionType.Sqrt,
                                 bias=eps_t[:], scale=1.0)
            nc.vector.reciprocal(out=alpha[:], in_=alpha[:])
            nc.vector.tensor_mul(alpha[:], alpha[:], gb[:, :, 0:1])
            bias_f = small.tile([128, MB, 1], F32)
            nc.vector.tensor_mul(bias_f[:], ge[:, :, 0:1], alpha[:])
            nc.vector.tensor_sub(bias_f[:], gb[:, :, 1:2], bias_f[:])

            # ---- apply BN + residual + relu, write out ----
            for mb in range(MB):
                flat_o = out_sb[:, mb].rearrange("p a b c -> p (a b c)")
                flat_x = x_res[:, mb].rearrange("p a b c -> p (a b c)")
                nc.vector.scalar_tensor_tensor(
                    out=flat_o, in0=flat_o, scalar=alpha[:, mb, :],
                    in1=flat_x, op0=mybir.AluOpType.mult,
                    op1=mybir.AluOpType.add)
                nc.scalar.activation(out=flat_o, in_=flat_o,
                                     func=mybir.ActivationFunctionType.Relu,
                                     bias=bias_f[:, mb, :], scale=1.0)
                for b in range(BLOC):
                    nc.sync.dma_start(
                        out=out_cbhw[mb * 128:(mb + 1) * 128, b, :],
                        in_=out_sb[:, mb, b].rearrange("p a b -> p (a b)"))

    nc.compile()
    return nc


_CACHE = {}


def kernel(x, w_conv1, w_center, b_center, gamma, beta):
    """Full-input entry point; shards batch across 8 NeuronCores."""
    x = np.ascontiguousarray(np.asarray(x, np.float32))
    w_conv1 = np.asarray(w_conv1, np.float32)
    w_center = np.asarray(w_center, np.float32)
    gamma = np.ascontiguousarray(np.asarray(gamma, np.float32))
    beta = np.ascontiguousarray(np.asarray(beta, np.float32))

    if os.environ.get("BASS_TRACE"):
        _install_ntff_hook()

    if "nc" not in _CACHE:
        _CACHE["nc"] = build_program()
    nc = _CACHE["nc"]

    # host-side weight relayout (transpose to lhsT [k, c, p]; fold 1/|bin|)
    w1f = w_conv1.reshape(C, C, 12)
    w1t = (np.ascontiguousarray(w1f.transpose(2, 1, 0))
           / BIN_N[:, None, None]).astype(ml_dtypes.bfloat16)
    wct = np.ascontiguousarray(w_center[:, :, 0, 0].T).astype(ml_dtypes.bfloat16)
    xb = x.astype(ml_dtypes.bfloat16)

    in_maps = []
    for i in range(NCORES):
        in_maps.append({
            "x": np.ascontiguousarray(x[i * BLOC:(i + 1) * BLOC]),
            "xb": np.ascontiguousarray(xb[i * BLOC:(i + 1) * BLOC]),
            "w1t": w1t, "wct": wct, "gamma": gamma, "beta": beta,
        })
    res = run_bass_kernel_spmd(nc, in_maps, list(range(NCORES)))
    _CACHE["last_result"] = res
    out = np.concatenate([res.results[i]["out"] for i in range(NCORES)], axis=0)
    return out.astype(np.float32)


if __name__ == "__main__":
    rng = np.random.default_rng(0)
    inputs = {
        "x": rng.standard_normal((B, C, H, W)).astype(np.float32),
        "w_conv1": (rng.standard_normal((C, C, 4, 3)) * 0.02).astype(np.float32),
        "w_center": (rng.standard_normal((C, C, 1, 1)) * 0.05).astype(np.float32),
        "b_center": (rng.standard_normal((C,)) * 0.01).astype(np.float32),
        "gamma": np.ones(C, np.float32),
        "beta": np.zeros(C, np.float32),
    }
    out = kernel(**inputs)
    print("out", out.shape, out.dtype, float(np.abs(out).max()))


# revision 13
# speedup vs baseline: 1.1777x; 1.1777x over previous
"""Trainium2 Bass kernel for nn_BasicBlockLogS (log-polar pooling block).

Math: the reference module (log_pooling -> conv1(stride 4,3) + center 1x1 conv
+ bias -> training-mode BatchNorm -> relu(out + x)) collapses exactly into a
9x9 conv whose taps are partitioned into 12 log-polar bins (taps in a bin share
one weight matrix, scaled 1/|bin|) plus a center 1x1 matrix.  b_center cancels
inside BatchNorm.  Each bin is 1-2 rectangular blocks of taps, so the conv is
computed as 21 accumulated matmuls over [C=256] per output tile, with rhs =
horizontal/vertical run-sum images of x (computed once on the Vector engine and
shared by all output channels).

Sharding: pure data parallel, batch 32 -> 4 per core across 8 cores.  BN batch
stats (per-channel mean / E[x^2]) are all-reduced across cores on-device.

The conv datapath (x frames, run images, weights) is bf16: DVE tensor_tensor
gets 2x mode, and bf16 weights use fast-weight-load so LDWEIGHTS hides under
the matmul stream.  PSUM accumulation, BN statistics, the residual x, and the
final output stay fp32.
"""

import os
import sys
import types
import numpy as np
from contextlib import ExitStack

for _p in ("/opt/trn_rl_repo",):
    if _p not in sys.path:
        sys.path.insert(0, _p)

import ml_dtypes
import concourse.bass as bass
import concourse.tile as tile
from concourse import bacc, mybir
from concourse.bass_utils import run_bass_kernel_spmd

F32 = mybir.dt.float32
BF16 = mybir.dt.bfloat16

NCORES = 8
B, C, H, W = 32, 256, 28, 28
BLOC = B // NCORES            # 4 batch items per core
CB = 2                        # channel blocks of 128 (contraction)
MB = 2                        # output-channel blocks of 128
HHALF = 14                    # output rows per half-frame
FR = HHALF + 8                # padded rows per half-frame (22)
NT = HHALF * W                # N per matmul tile (392)
EPS = 1e-5

# log-polar bin sizes (taps per bin), bins k=0..11 (k = bh*3+bw order)
BIN_N = np.array([2, 1, 1, 2, 1, 1, 14, 11, 11, 14, 11, 11], np.float32)

# Segment table: (weight idx 0..12 [12=center], source, row offset, col offset)
# source: one of the run images, or 'xp' (padded x frame, col offset 4+dx).
# For a segment reading tensor G at vertical tap offset dy, local row = h'+ofs.
SEGS = [
    (9,  "v4L3", 1, 0),   # bin9  rows dy=-3..0,  cols [-4..-2]
    (9,  "L2",   0, 0),   # bin9  row  dy=-4,     cols [-4,-3]
    (10, "v2C3", 1, 0),   # bin10 rows dy=-3..-2, cols [-1..1]
    (10, "C5",   0, 0),   # bin10 row  dy=-4,     cols [-2..2]
    (11, "v3R3", 1, 0),   # bin11 rows dy=-3..-1, cols [2..4]
    (11, "R2",   0, 0),   # bin11 row  dy=-4,     cols [3,4]
    (6,  "v4R3", 4, 0),   # bin6  rows dy=0..3,   cols [2..4]
    (6,  "R2",   8, 0),   # bin6  row  dy=+4,     cols [3,4]
    (7,  "v2C3", 6, 0),   # bin7  rows dy=+2..+3, cols [-1..1]
    (7,  "C5",   8, 0),   # bin7  row  dy=+4,     cols [-2..2]
    (8,  "v3L3", 5, 0),   # bin8  rows dy=+1..+3, cols [-4..-2]
    (8,  "L2",   8, 0),   # bin8  row  dy=+4,     cols [-4,-3]
    (0,  "v2x",  4, 1),   # bin0  (0,+1)+(1,+1)
    (1,  "xp",   5, 0),   # bin1  (1,0)
    (2,  "xp",   5, -1),  # bin2  (1,-1)
    (3,  "v2x",  3, -1),  # bin3  (-1,-1)+(0,-1)
    (4,  "xp",   3, 0),   # bin4  (-1,0)
    (5,  "xp",   3, 1),   # bin5  (-1,1)
    (12, "xp",   4, 0),   # center 1x1
]


def _install_ntff_hook():
    """Register the axon NTFF profiling hook (absent antenv.axon_hooks shim)."""
    if "antenv.axon_hooks" in sys.modules:
        return
    mod = types.ModuleType("antenv.axon_hooks")
    mod._hook = None
    mod.set_axon_ntff_profile_hook = lambda h: setattr(mod, "_hook", h)
    mod.get_axon_ntff_profile_hook = lambda: mod._hook
    sys.modules["antenv.axon_hooks"] = mod
    try:
        from trn_agent_boot.trn_boot import _ntff_profile_via_ctypes
        mod.set_axon_ntff_profile_hook(
            _ntff_profile_via_ctypes("/opt/axon/libaxon_pjrt.so"))
    except Exception:
        pass


def build_program():
    nc = bacc.Bacc("TRN2", target_bir_lowering=False, debug=False,
                   num_devices=NCORES)

    x_in = nc.dram_tensor("x", [BLOC, C, H, W], F32, kind="ExternalInput").ap()
    xb_in = nc.dram_tensor("xb", [BLOC, C, H, W], BF16, kind="ExternalInput").ap()
    w1_in = nc.dram_tensor("w1t", [12, C, C], BF16, kind="ExternalInput").ap()
    wc_in = nc.dram_tensor("wct", [C, C], BF16, kind="ExternalInput").ap()
    g_in = nc.dram_tensor("gamma", [C], F32, kind="ExternalInput").ap()
    bt_in = nc.dram_tensor("beta", [C], F32, kind="ExternalInput").ap()
    out_d = nc.dram_tensor("out", [BLOC, C, H, W], F32, kind="ExternalOutput").ap()

    cc_in_d = nc.dram_tensor("cc_in", [128, 2 * MB], F32)
    cc_out_d = nc.dram_tensor("cc_out", [128, 2 * MB], F32, addr_space="Shared")

    # DRAM views with channels on partitions
    x_cbhw = x_in.rearrange("b c h w -> c b h w")
    xb_cbhw = xb_in.rearrange("b c h w -> c b h w")
    out_cbhw = out_d.rearrange("b c h w -> c b (h w)")

    with tile.TileContext(nc) as tc:
        with ExitStack() as ctx:
            persist = ctx.enter_context(tc.tile_pool(name="persist", bufs=1))
            stage = ctx.enter_context(tc.tile_pool(name="stage", bufs=2))
            trans = ctx.enter_context(tc.tile_pool(name="trans", bufs=7))
            psum = ctx.enter_context(tc.tile_pool(name="psum", bufs=4, space="PSUM"))
            small = ctx.enter_context(tc.tile_pool(name="small", bufs=1))

            # ---- persistent tiles ----
            w_all = persist.tile([128, CB, 13, C], BF16)     # lhsT: [c, p] per k
            gb = persist.tile([128, MB, 2], F32)             # gamma, beta
            out_sb = persist.tile([128, MB, BLOC, 2, NT], F32)
            x_res = persist.tile([128, MB, BLOC, 2, NT], F32)
            sb_stats = persist.tile([128, MB, BLOC * 2, 6], F32)
            eps_t = small.tile([128, 1], F32)
            nc.vector.memset(eps_t[:], EPS)

            # x frame slots (one per half): zero once; per-group DMA rewrites
            # only the interior, pads stay zero.
            xp_slots = [persist.tile([128, CB, FR, 36], BF16, name=f"xps{i}")
                        for i in range(2)]
            for i in range(2):
                nc.gpsimd.memset(xp_slots[i][:], 0.0)

            # emit the first two groups' x DMAs before the weight DMAs so the
            # Vector engine's run-sum chain starts as early as possible
            def emit_x_dma(b, half):
                H0 = half * HHALF
                xr0 = max(H0 - 4, 0)
                xr1 = min(H0 + FR - 4, H) - 1
                nxr = xr1 - xr0 + 1
                l0 = xr0 + 4 - H0
                for cb in range(CB):
                    nc.sync.dma_start(
                        out=xp_slots[half][:, cb, l0:l0 + nxr, 4:32],
                        in_=xb_cbhw[cb * 128:(cb + 1) * 128, b,
                                    xr0:xr0 + nxr, :])

            emit_x_dma(0, 0)
            emit_x_dma(0, 1)

            # ---- weights in ----
            for k in range(12):
                for cb in range(CB):
                    nc.sync.dma_start(
                        out=w_all[:, cb, k, :],
                        in_=w1_in[k, cb * 128:(cb + 1) * 128, :])
            for cb in range(CB):
                nc.sync.dma_start(
                    out=w_all[:, cb, 12, :],
                    in_=wc_in[cb * 128:(cb + 1) * 128, :])
            nc.sync.dma_start(out=gb[:, :, 0],
                              in_=g_in.rearrange("(cb c) -> c cb", c=128))
            nc.sync.dma_start(out=gb[:, :, 1],
                              in_=bt_in.rearrange("(cb c) -> c cb", c=128))

            # warm up the collective path early so the real stats AllReduce
            # doesn't pay ncfw comm-init; overlaps with the matmul phase
            cc_w_in = nc.dram_tensor("cc_w_in", [128, 1], F32)
            cc_w_out = nc.dram_tensor("cc_w_out", [128, 1], F32,
                                      addr_space="Shared")
            nc.sync.dma_start(out=cc_w_in.ap(), in_=eps_t[:])
            nc.gpsimd.collective_compute(
                "AllReduce", mybir.AluOpType.add,
                replica_groups=[list(range(NCORES))],
                ins=[cc_w_in.ap()], outs=[cc_w_out.ap()])

            # ---- main loop over (batch item, half) groups ----
            for b in range(BLOC):
                for half in range(2):
                    g = b * 2 + half
                    H0 = half * HHALF
                    # padded frame rows H0..H0+21; interior x rows:
                    xr0 = max(H0 - 4, 0)            # first x row
                    xr1 = min(H0 + FR - 4, H) - 1   # last x row (incl)
                    nxr = xr1 - xr0 + 1             # 18
                    l0 = xr0 + 4 - H0               # local row of first x row

                    xp = xp_slots[half]
                    if b > 0:
                        emit_x_dma(b, half)

                    # ---- horizontal run sums (DVE) ----
                    def st(tag):
                        return stage.tile([128, CB, FR, W], BF16, name=tag, tag=tag)

                    def tr(tag):
                        return trans.tile([128, CB, FR, W], BF16, name=tag, tag="tmp")

                    L2 = st("L2")
                    nc.vector.tensor_add(L2[:], xp[:, :, :, 0:28], xp[:, :, :, 1:29])
                    L3 = tr("L3")
                    nc.vector.tensor_add(L3[:], L2[:], xp[:, :, :, 2:30])
                    C3 = tr("C3")
                    nc.vector.tensor_add(C3[:], xp[:, :, :, 3:31], xp[:, :, :, 4:32])
                    nc.vector.tensor_add(C3[:], C3[:], xp[:, :, :, 5:33])
                    C5 = st("C5")
                    nc.vector.tensor_add(C5[:], C3[:], xp[:, :, :, 2:30])
                    nc.vector.tensor_add(C5[:], C5[:], xp[:, :, :, 6:34])
                    R2 = st("R2")
                    nc.vector.tensor_add(R2[:], xp[:, :, :, 7:35], xp[:, :, :, 8:36])
                    R3 = tr("R3")
                    nc.vector.tensor_add(R3[:], R2[:], xp[:, :, :, 6:34])

                    # ---- vertical run sums ----
                    v2C3 = st("v2C3")
                    nc.vector.tensor_add(v2C3[:, :, 0:FR - 1, :],
                                         C3[:, :, 0:FR - 1, :], C3[:, :, 1:FR, :])
                    v2L3 = tr("v2L3")
                    nc.vector.tensor_add(v2L3[:, :, 0:FR - 1, :],
                                         L3[:, :, 0:FR - 1, :], L3[:, :, 1:FR, :])
                    v3L3 = st("v3L3")
                    nc.vector.tensor_add(v3L3[:, :, 0:FR - 2, :],
                                         v2L3[:, :, 0:FR - 2, :], L3[:, :, 2:FR, :])
                    v4L3 = st("v4L3")
                    nc.vector.tensor_add(v4L3[:, :, 0:FR - 3, :],
                                         v2L3[:, :, 0:FR - 3, :],
                                         v2L3[:, :, 2:FR - 1, :])
                    v2R3 = tr("v2R3")
                    nc.vector.tensor_add(v2R3[:, :, 0:FR - 1, :],
                                         R3[:, :, 0:FR - 1, :], R3[:, :, 1:FR, :])
                    v3R3 = st("v3R3")
                    nc.vector.tensor_add(v3R3[:, :, 0:FR - 2, :],
                                         v2R3[:, :, 0:FR - 2, :], R3[:, :, 2:FR, :])
                    v4R3 = st("v4R3")
                    nc.vector.tensor_add(v4R3[:, :, 0:FR - 3, :],
                                         v2R3[:, :, 0:FR - 3, :],
                                         v2R3[:, :, 2:FR - 1, :])

                    v2x = stage.tile([128, CB, FR, 36], BF16, name="v2x",
                                     tag="v2x")
                    nc.vector.tensor_add(v2x[:, :, 0:FR - 1, :],
                                         xp[:, :, 0:FR - 1, :],
                                         xp[:, :, 1:FR, :])

                    runs = {"L2": L2, "C5": C5, "R2": R2, "v2C3": v2C3,
                            "v3L3": v3L3, "v4L3": v4L3, "v3R3": v3R3,
                            "v4R3": v4R3, "xp": xp, "v2x": v2x}

                    # fp32 x interior for the residual (emitted after the runs
                    # so these DMAs don't compete with the critical path)
                    for cb in range(CB):
                        nc.sync.dma_start(
                            out=x_res[:, cb, b, half, :],
                            in_=x_cbhw[cb * 128:(cb + 1) * 128, b,
                                       H0:H0 + HHALF, :])

                    # ---- 21 segments x 2 cblk accumulated matmuls ----
                    for mb in range(MB):
                        ps = psum.tile([128, NT], F32, tag="ps")
                        n_mm = len(SEGS) * CB
                        si = 0
                        for (wi, src, ro, co) in SEGS:
                            tsrc = runs[src]
                            for cb in range(CB):
                                if src in ("xp", "v2x"):
                                    rhs = tsrc[:, cb, ro:ro + HHALF,
                                               4 + co:4 + co + W]
                                else:
                                    rhs = tsrc[:, cb, ro:ro + HHALF, 0:W]
                                nc.tensor.matmul(
                                    ps[:],
                                    lhsT=w_all[:, cb, wi,
                                               mb * 128:(mb + 1) * 128],
                                    rhs=rhs,
                                    start=(si == 0), stop=(si == n_mm - 1))
                                si += 1
                        # per-tile batch-norm partial stats + copy out of PSUM
                        nc.vector.bn_stats(out=sb_stats[:, mb, g, :], in_=ps[:])
                        nc.scalar.copy(out=out_sb[:, mb, b, half, :], in_=ps[:])

            # ---- local stats -> (mean, E[x^2]) -> AllReduce ----
            mv = small.tile([128, MB, 2], F32)
            for mb in range(MB):
                nc.vector.bn_aggr(out=mv[:, mb, :], in_=sb_stats[:, mb, :, :])
            pack = small.tile([128, MB, 2], F32)
            nc.vector.tensor_copy(pack[:, :, 0:1], mv[:, :, 0:1])
            nc.vector.tensor_mul(pack[:, :, 1:2], mv[:, :, 0:1], mv[:, :, 0:1])
            nc.vector.tensor_add(pack[:, :, 1:2], pack[:, :, 1:2], mv[:, :, 1:2])
            nc.sync.dma_start(out=cc_in_d.ap(),
                              in_=pack[:].rearrange("p a b -> p (a b)"))
            nc.gpsimd.collective_compute(
                "AllReduce", mybir.AluOpType.add,
                replica_groups=[list(range(NCORES))],
                ins=[cc_in_d.ap()], outs=[cc_out_d.ap()])
            glob = small.tile([128, MB, 2], F32)
            nc.sync.dma_start(out=glob[:].rearrange("p a b -> p (a b)"),
                              in_=cc_out_d.ap())

            # global mean / var -> alpha, bias
            ge = small.tile([128, MB, 2], F32)
            nc.vector.tensor_scalar_mul(ge[:], glob[:], 1.0 / NCORES)
            var_g = small.tile([128, MB, 1], F32)
            nc.vector.tensor_mul(var_g[:], ge[:, :, 0:1], ge[:, :, 0:1])
            nc.vector.tensor_sub(var_g[:], ge[:, :, 1:2], var_g[:])
            alpha = small.tile([128, MB, 1], F32)
            nc.scalar.activation(out=alpha[:], in_=var_g[:],
                                 func=mybir.ActivationFunctionType.Sqrt,
                                 bias=eps_t[:], scale=1.0)
            nc.vector.reciprocal(out=alpha[:], in_=alpha[:])
            nc.vector.tensor_mul(alpha[:], alpha[:], gb[:, :, 0:1])
            bias_f = small.tile([128, MB, 1], F32)
            nc.vector.tensor_mul(bias_f[:], ge[:, :, 0:1], alpha[:])
            nc.vector.tensor_sub(bias_f[:], gb[:, :, 1:2], bias_f[:])

            # ---- apply BN + residual + relu, write out ----
            # chunked by (mb, b) so DVE -> ACT -> DMA pipeline per chunk
            for mb in range(MB):
                for b in range(BLOC):
                    flat_o = out_sb[:, mb, b].rearrange("p a b -> p (a b)")
                    flat_x = x_res[:, mb, b].rearrange("p a b -> p (a b)")
                    nc.vector.scalar_tensor_tensor(
                        out=flat_o, in0=flat_o, scalar=alpha[:, mb, :],
                        in1=flat_x, op0=mybir.AluOpType.mult,
                        op1=mybir.AluOpType.add)
                    nc.scalar.activation(out=flat_o, in_=flat_o,
                                         func=mybir.ActivationFunctionType.Relu,
                                         bias=bias_f[:, mb, :], scale=1.0)
                    nc.sync.dma_start(
                        out=out_cbhw[mb * 128:(mb + 1) * 128, b, :],
                        in_=flat_o)

    nc.compile()
    return nc


_CACHE = {}


def kernel(x, w_conv1, w_center, b_center, gamma, beta):
    """Full-input entry point; shards batch across 8 NeuronCores."""
    x = np.ascontiguousarray(np.asarray(x, np.float32))
    w_conv1 = np.asarray(w_conv1, np.float32)
    w_center = np.asarray(w_center, np.float32)
    gamma = np.ascontiguousarray(np.asarray(gamma, np.float32))
    beta = np.ascontiguousarray(np.asarray(beta, np.float32))

    if os.environ.get("BASS_TRACE"):
        _install_ntff_hook()

    if "nc" not in _CACHE:
        _CACHE["nc"] = build_program()
    nc = _CACHE["nc"]

    # host-side weight relayout (transpose to lhsT [k, c, p]; fold 1/|bin|)
    w1f = w_conv1.reshape(C, C, 12)
    w1t = (np.ascontiguousarray(w1f.transpose(2, 1, 0))
           / BIN_N[:, None, None]).astype(ml_dtypes.bfloat16)
    wct = np.ascontiguousarray(w_center[:, :, 0, 0].T).astype(ml_dtypes.bfloat16)
    xb = x.astype(ml_dtypes.bfloat16)

    in_maps = []
    for i in range(NCORES):
        in_maps.append({
            "x": np.ascontiguousarray(x[i * BLOC:(i + 1) * BLOC]),
            "xb": np.ascontiguousarray(xb[i * BLOC:(i + 1) * BLOC]),
            "w1t": w1t, "wct": wct, "gamma": gamma, "beta": beta,
        })
    res = run_bass_kernel_spmd(nc, in_maps, list(range(NCORES)))
    _CACHE["last_result"] = res
    out = np.concatenate([res.results[i]["out"] for i in range(NCORES)], axis=0)
    return out.astype(np.float32)


if __name__ == "__main__":
    rng = np.random.default_rng(0)
    inputs = {
        "x": rng.standard_normal((B, C, H, W)).astype(np.float32),
        "w_conv1": (rng.standard_normal((C, C, 4, 3)) * 0.02).astype(np.float32),
        "w_center": (rng.standard_normal((C, C, 1, 1)) * 0.05).astype(np.float32),
        "b_center": (rng.standard_normal((C,)) * 0.01).astype(np.float32),
        "gamma": np.ones(C, np.float32),
        "beta": np.zeros(C, np.float32),
    }
    out = kernel(**inputs)
    print("out", out.shape, out.dtype, float(np.abs(out).max()))


# revision 17
# speedup vs baseline: 1.4388x; 1.2218x over previous
"""Trainium2 Bass kernel for nn_BasicBlockLogS (log-polar pooling block).

Math: the reference module (log_pooling -> conv1(stride 4,3) + center 1x1 conv
+ bias -> training-mode BatchNorm -> relu(out + x)) collapses exactly into a
9x9 conv whose taps are partitioned into 12 log-polar bins (taps in a bin share
one weight matrix, scaled 1/|bin|) plus a center 1x1 matrix.  b_center cancels
inside BatchNorm.  Each bin is 1-2 rectangular blocks of taps, so the conv is
computed as 21 accumulated matmuls over [C=256] per output tile, with rhs =
horizontal/vertical run-sum images of x (computed once on the Vector engine and
shared by all output channels).

Sharding: pure data parallel, batch 32 -> 4 per core across 8 cores.  BN batch
stats (per-channel mean / E[x^2]) are all-reduced across cores on-device.

The conv datapath (x frames, run images, weights) is bf16: DVE tensor_tensor
gets 2x mode, and bf16 weights use fast-weight-load so LDWEIGHTS hides under
the matmul stream.  PSUM accumulation, BN statistics, the residual x, and the
final output stay fp32.
"""

import os
import sys
import types
import numpy as np
from contextlib import ExitStack

for _p in ("/opt/trn_rl_repo",):
    if _p not in sys.path:
        sys.path.insert(0, _p)

import ml_dtypes
import concourse.bass as bass
import concourse.tile as tile
from concourse import bacc, mybir
from concourse.bass_utils import run_bass_kernel_spmd

F32 = mybir.dt.float32
BF16 = mybir.dt.bfloat16

NCORES = 8
B, C, H, W = 32, 256, 28, 28
BLOC = B // NCORES            # 4 batch items per core
CB = 2                        # channel blocks of 128 (contraction)
MB = 2                        # output-channel blocks of 128
HHALF = 14                    # output rows per matmul N-tile
FR = 36                       # padded rows per item frame
NT = HHALF * W                # N per matmul tile (392)
EPS = 1e-5

# log-polar bin sizes (taps per bin), bins k=0..11 (k = bh*3+bw order)
BIN_N = np.array([2, 1, 1, 2, 1, 1, 14, 11, 11, 14, 11, 11], np.float32)

# Segment table: (weight idx 0..12 [12=center], source, row offset, col offset)
# For a segment reading tensor G with row anchor ofs, rhs rows = h+ofs.
# Ordered so shallow-dependency sources (xp, v2x) come first: the PE can
# start them while the Vector engine is still building the deeper run sums.
# T9/T8/T6/T11 are the fully-merged big-bin tensors (one matmul per bin).
SEGS = [
    (12, "xp",   4, 0),   # center 1x1
    (1,  "xp",   5, 0),   # bin1  (1,0)
    (2,  "xp",   5, -1),  # bin2  (1,-1)
    (4,  "xp",   3, 0),   # bin4  (-1,0)
    (5,  "xp",   3, 1),   # bin5  (-1,1)
    (0,  "v2x",  4, 1),   # bin0  (0,+1)+(1,+1)
    (3,  "v2x",  3, -1),  # bin3  (-1,-1)+(0,-1)
    (10, "v2C3", 1, 0),   # bin10 rows dy=-3..-2, cols [-1..1]
    (10, "C5",   0, 0),   # bin10 row  dy=-4,     cols [-2..2]
    (7,  "v2C3", 6, 0),   # bin7  rows dy=+2..+3, cols [-1..1]
    (7,  "C5",   8, 0),   # bin7  row  dy=+4,     cols [-2..2]
    (9,  "T9",   0, 0),   # bin9 merged: v4L3[r+1] + L2[r]
    (8,  "T8",   0, 0),   # bin8 merged: v3L3[r+5] + L2[r+8]
    (6,  "T6",   0, 0),   # bin6 merged: v4R3[r+4] + R2[r+8]
    (11, "T11",  0, 0),   # bin11 merged: v3R3[r+1] + R2[r]
]
# weight-load order: first-used first
WORDER = [12, 1, 2, 4, 5, 0, 3, 10, 7, 9, 8, 6, 11]


def _install_ntff_hook():
    """Register the axon NTFF profiling hook (absent antenv.axon_hooks shim)."""
    if "antenv.axon_hooks" in sys.modules:
        return
    mod = types.ModuleType("antenv.axon_hooks")
    mod._hook = None
    mod.set_axon_ntff_profile_hook = lambda h: setattr(mod, "_hook", h)
    mod.get_axon_ntff_profile_hook = lambda: mod._hook
    sys.modules["antenv.axon_hooks"] = mod
    try:
        from trn_agent_boot.trn_boot import _ntff_profile_via_ctypes
        mod.set_axon_ntff_profile_hook(
            _ntff_profile_via_ctypes("/opt/axon/libaxon_pjrt.so"))
    except Exception:
        pass


def build_program():
    nc = bacc.Bacc("TRN2", target_bir_lowering=False, debug=False,
                   num_devices=NCORES)

    x_in = nc.dram_tensor("x", [BLOC, C, H, W], F32, kind="ExternalInput").ap()
    xb_in = nc.dram_tensor("xb", [BLOC, C, H, W], BF16, kind="ExternalInput").ap()
    w1_in = nc.dram_tensor("w1t", [12, C, C], BF16, kind="ExternalInput").ap()
    wc_in = nc.dram_tensor("wct", [C, C], BF16, kind="ExternalInput").ap()
    g_in = nc.dram_tensor("gamma", [C], F32, kind="ExternalInput").ap()
    bt_in = nc.dram_tensor("beta", [C], F32, kind="ExternalInput").ap()
    out_d = nc.dram_tensor("out", [BLOC, C, H, W], F32, kind="ExternalOutput").ap()

    cc_in_d = nc.dram_tensor("cc_in", [128, 2 * MB], F32)
    cc_out_d = nc.dram_tensor("cc_out", [128, 2 * MB], F32, addr_space="Shared")

    # DRAM views with channels on partitions
    x_cbhw = x_in.rearrange("b c h w -> c b h w")
    xb_cbhw = xb_in.rearrange("b c h w -> c b h w")
    out_cbhw = out_d.rearrange("b c h w -> c b (h w)")

    with tile.TileContext(nc) as tc:
        with ExitStack() as ctx:
            persist = ctx.enter_context(tc.tile_pool(name="persist", bufs=1))
            stage = ctx.enter_context(tc.tile_pool(name="stage", bufs=2))
            trans = ctx.enter_context(tc.tile_pool(name="trans", bufs=11))
            psum = ctx.enter_context(tc.tile_pool(name="psum", bufs=6, space="PSUM"))
            small = ctx.enter_context(tc.tile_pool(name="small", bufs=1))

            # ---- persistent tiles ----
            w_all = persist.tile([128, CB, 13, C], BF16)     # lhsT: [c, p] per k
            gb = persist.tile([128, MB, 2], F32)             # gamma, beta
            out_sb = persist.tile([128, MB, BLOC, 2, NT], F32)
            x_res = persist.tile([128, MB, BLOC, 2, NT], F32)
            sb_stats = persist.tile([128, MB, BLOC * 2, 6], F32)
            eps_t = small.tile([128, 1], F32)
            nc.vector.memset(eps_t[:], EPS)

            # x frame slots (one per item parity): zero once; per-item DMA
            # rewrites only the interior, pads stay zero.
            xp_slots = [persist.tile([128, CB, FR, 36], BF16, name=f"xps{i}")
                        for i in range(2)]
            for i in range(2):
                nc.gpsimd.memset(xp_slots[i][:], 0.0)

            # emit the first two items' x DMAs before the weight DMAs so the
            # Vector engine's run-sum chain starts as early as possible
            def emit_x_dma(b):
                for cb in range(CB):
                    nc.sync.dma_start(
                        out=xp_slots[b % 2][:, cb, 4:32, 4:32],
                        in_=xb_cbhw[cb * 128:(cb + 1) * 128, b, :, :])

            emit_x_dma(0)
            emit_x_dma(1)

            # ---- weights in (first-used first) ----
            for k in WORDER:
                src = wc_in if k == 12 else w1_in[k]
                for cb in range(CB):
                    nc.sync.dma_start(
                        out=w_all[:, cb, k, :],
                        in_=src[cb * 128:(cb + 1) * 128, :])
            nc.sync.dma_start(out=gb[:, :, 0],
                              in_=g_in.rearrange("(cb c) -> c cb", c=128))
            nc.sync.dma_start(out=gb[:, :, 1],
                              in_=bt_in.rearrange("(cb c) -> c cb", c=128))

            # warm up the collective path early so the real stats AllReduce
            # doesn't pay ncfw comm-init; overlaps with the matmul phase
            cc_w_in = nc.dram_tensor("cc_w_in", [128, 1], F32)
            cc_w_out = nc.dram_tensor("cc_w_out", [128, 1], F32,
                                      addr_space="Shared")
            nc.sync.dma_start(out=cc_w_in.ap(), in_=eps_t[:])
            nc.gpsimd.collective_compute(
                "AllReduce", mybir.AluOpType.add,
                replica_groups=[list(range(NCORES))],
                ins=[cc_w_in.ap()], outs=[cc_w_out.ap()])

            # ---- main loop over batch items ----
            for b in range(BLOC):
                xp = xp_slots[b % 2]
                if b > 1:
                    emit_x_dma(b)

                def st(tag):
                    return stage.tile([128, CB, FR, W], BF16, name=tag, tag=tag)

                def tr(tag):
                    return trans.tile([128, CB, FR, W], BF16, name=tag, tag="tmp")

                # v2x first: unblocks the v2x segments right after xp lands
                v2x = stage.tile([128, CB, FR, 36], BF16, name="v2x", tag="v2x")
                nc.vector.tensor_add(v2x[:, :, 0:FR - 1, :],
                                     xp[:, :, 0:FR - 1, :], xp[:, :, 1:FR, :])

                # ---- horizontal run sums ----
                L2 = tr("L2")
                nc.vector.tensor_add(L2[:], xp[:, :, :, 0:28], xp[:, :, :, 1:29])
                R2 = tr("R2")
                nc.vector.tensor_add(R2[:], xp[:, :, :, 7:35], xp[:, :, :, 8:36])
                C3 = tr("C3")
                nc.vector.tensor_add(C3[:], xp[:, :, :, 3:31], xp[:, :, :, 4:32])
                nc.vector.tensor_add(C3[:], C3[:], xp[:, :, :, 5:33])
                v2C3 = st("v2C3")
                nc.vector.tensor_add(v2C3[:, :, 0:FR - 1, :],
                                     C3[:, :, 0:FR - 1, :], C3[:, :, 1:FR, :])
                C5 = st("C5")
                nc.vector.tensor_add(C5[:], C3[:], xp[:, :, :, 2:30])
                nc.vector.tensor_add(C5[:], C5[:], xp[:, :, :, 6:34])

                # ---- L side: v-runs + merged bins 9, 8 ----
                L3 = tr("L3")
                nc.vector.tensor_add(L3[:], L2[:], xp[:, :, :, 2:30])
                v2L3 = tr("v2L3")
                nc.vector.tensor_add(v2L3[:, :, 0:FR - 1, :],
                                     L3[:, :, 0:FR - 1, :], L3[:, :, 1:FR, :])
                v4L3 = tr("v4L3")
                nc.vector.tensor_add(v4L3[:, :, 0:FR - 3, :],
                                     v2L3[:, :, 0:FR - 3, :],
                                     v2L3[:, :, 2:FR - 1, :])
                T9 = st("T9")
                nc.vector.tensor_add(T9[:, :, 0:28, :], v4L3[:, :, 1:29, :],
                                     L2[:, :, 0:28, :])
                v3L3 = tr("v3L3")
                nc.vector.tensor_add(v3L3[:, :, 0:FR - 2, :],
                                     v2L3[:, :, 0:FR - 2, :], L3[:, :, 2:FR, :])
                T8 = st("T8")
                nc.vector.tensor_add(T8[:, :, 0:28, :], v3L3[:, :, 5:33, :],
                                     L2[:, :, 8:36, :])

                # ---- R side: v-runs + merged bins 6, 11 ----
                R3 = tr("R3")
                nc.vector.tensor_add(R3[:], R2[:], xp[:, :, :, 6:34])
                v2R3 = tr("v2R3")
                nc.vector.tensor_add(v2R3[:, :, 0:FR - 1, :],
                                     R3[:, :, 0:FR - 1, :], R3[:, :, 1:FR, :])
                v4R3 = tr("v4R3")
                nc.vector.tensor_add(v4R3[:, :, 0:FR - 3, :],
                                     v2R3[:, :, 0:FR - 3, :],
                                     v2R3[:, :, 2:FR - 1, :])
                T6 = st("T6")
                nc.vector.tensor_add(T6[:, :, 0:28, :], v4R3[:, :, 4:32, :],
                                     R2[:, :, 8:36, :])
                v3R3 = tr("v3R3")
                nc.vector.tensor_add(v3R3[:, :, 0:FR - 2, :],
                                     v2R3[:, :, 0:FR - 2, :], R3[:, :, 2:FR, :])
                T11 = st("T11")
                nc.vector.tensor_add(T11[:, :, 0:28, :], v3R3[:, :, 1:29, :],
                                     R2[:, :, 0:28, :])

                runs = {"C5": C5, "v2C3": v2C3, "T9": T9, "T8": T8,
                        "T6": T6, "T11": T11, "xp": xp, "v2x": v2x}

                # fp32 x for the residual (after the runs: keeps DMA queues
                # clear for the critical path)
                for cb in range(CB):
                    nc.sync.dma_start(
                        out=x_res[:, cb, b].rearrange("p a b -> p (a b)"),
                        in_=x_cbhw[cb * 128:(cb + 1) * 128, b, :, :]
                        .rearrange("p a b -> p (a b)"))

                # ---- 15 segments x 2 cblk accumulated matmuls ----
                for mb in range(MB):
                    for half in range(2):
                        g = b * 2 + half
                        ps = psum.tile([128, NT], F32, name="ps", tag="ps")
                        n_mm = len(SEGS) * CB
                        si = 0
                        for (wi, src, ro, co) in SEGS:
                            tsrc = runs[src]
                            r0 = ro + HHALF * half
                            for cb in range(CB):
                                if src in ("xp", "v2x"):
                                    rhs = tsrc[:, cb, r0:r0 + HHALF,
                                               4 + co:4 + co + W]
                                else:
                                    rhs = tsrc[:, cb, r0:r0 + HHALF, 0:W]
                                nc.tensor.matmul(
                                    ps[:],
                                    lhsT=w_all[:, cb, wi,
                                               mb * 128:(mb + 1) * 128],
                                    rhs=rhs,
                                    start=(si == 0), stop=(si == n_mm - 1))
                                si += 1
                        # per-tile batch-norm partial stats + copy off PSUM
                        nc.vector.bn_stats(out=sb_stats[:, mb, g, :], in_=ps[:])
                        nc.scalar.copy(out=out_sb[:, mb, b, half, :], in_=ps[:])

            # ---- local stats -> (mean, E[x^2]) -> AllReduce ----
            mv = small.tile([128, MB, 2], F32)
            for mb in range(MB):
                nc.vector.bn_aggr(out=mv[:, mb, :], in_=sb_stats[:, mb, :, :])
            pack = small.tile([128, MB, 2], F32)
            nc.vector.tensor_copy(pack[:, :, 0:1], mv[:, :, 0:1])
            nc.vector.tensor_mul(pack[:, :, 1:2], mv[:, :, 0:1], mv[:, :, 0:1])
            nc.vector.tensor_add(pack[:, :, 1:2], pack[:, :, 1:2], mv[:, :, 1:2])
            nc.sync.dma_start(out=cc_in_d.ap(),
                              in_=pack[:].rearrange("p a b -> p (a b)"))
            nc.gpsimd.collective_compute(
                "AllReduce", mybir.AluOpType.add,
                replica_groups=[list(range(NCORES))],
                ins=[cc_in_d.ap()], outs=[cc_out_d.ap()])
            glob = small.tile([128, MB, 2], F32)
            nc.sync.dma_start(out=glob[:].rearrange("p a b -> p (a b)"),
                              in_=cc_out_d.ap())

            # global mean / var -> alpha, bias
            ge = small.tile([128, MB, 2], F32)
            nc.vector.tensor_scalar_mul(ge[:], glob[:], 1.0 / NCORES)
            var_g = small.tile([128, MB, 1], F32)
            nc.vector.tensor_mul(var_g[:], ge[:, :, 0:1], ge[:, :, 0:1])
            nc.vector.tensor_sub(var_g[:], ge[:, :, 1:2], var_g[:])
            alpha = small.tile([128, MB, 1], F32)
            nc.scalar.activation(out=alpha[:], in_=var_g[:],
                                 func=mybir.ActivationFunctionType.Sqrt,
                                 bias=eps_t[:], scale=1.0)
            nc.vector.reciprocal(out=alpha[:], in_=alpha[:])
            nc.vector.tensor_mul(alpha[:], alpha[:], gb[:, :, 0:1])
            bias_f = small.tile([128, MB, 1], F32)
            nc.vector.tensor_mul(bias_f[:], ge[:, :, 0:1], alpha[:])
            nc.vector.tensor_sub(bias_f[:], gb[:, :, 1:2], bias_f[:])

            # ---- apply BN + residual + relu, write out ----
            # chunked by (mb, b) so DVE -> ACT -> DMA pipeline per chunk
            for mb in range(MB):
                for b in range(BLOC):
                    flat_o = out_sb[:, mb, b].rearrange("p a b -> p (a b)")
                    flat_x = x_res[:, mb, b].rearrange("p a b -> p (a b)")
                    nc.vector.scalar_tensor_tensor(
                        out=flat_o, in0=flat_o, scalar=alpha[:, mb, :],
                        in1=flat_x, op0=mybir.AluOpType.mult,
                        op1=mybir.AluOpType.add)
                    nc.scalar.activation(out=flat_o, in_=flat_o,
                                         func=mybir.ActivationFunctionType.Relu,
                                         bias=bias_f[:, mb, :], scale=1.0)
                    nc.sync.dma_start(
                        out=out_cbhw[mb * 128:(mb + 1) * 128, b, :],
                        in_=flat_o)

    nc.compile()
    return nc


_CACHE = {}


def kernel(x, w_conv1, w_center, b_center, gamma, beta):
    """Full-input entry point; shards batch across 8 NeuronCores."""
    x = np.ascontiguousarray(np.asarray(x, np.float32))
    w_conv1 = np.asarray(w_conv1, np.float32)
    w_center = np.asarray(w_center, np.float32)
    gamma = np.ascontiguousarray(np.asarray(gamma, np.float32))
    beta = np.ascontiguousarray(np.asarray(beta, np.float32))

    if os.environ.get("BASS_TRACE"):
        _install_ntff_hook()

    if "nc" not in _CACHE:
        _CACHE["nc"] = build_program()
    nc = _CACHE["nc"]

    # host-side weight relayout (transpose to lhsT [k, c, p]; fold 1/|bin|)
    w1f = w_conv1.reshape(C, C, 12)
    w1t = (np.ascontiguousarray(w1f.transpose(2, 1, 0))
           / BIN_N[:, None, None]).astype(ml_dtypes.bfloat16)
    wct = np.ascontiguousarray(w_center[:, :, 0, 0].T).astype(ml_dtypes.bfloat16)
    xb = x.astype(ml_dtypes.bfloat16)

    in_maps = []
    for i in range(NCORES):
        in_maps.append({
            "x": np.ascontiguousarray(x[i * BLOC:(i + 1) * BLOC]),
            "xb": np.ascontiguousarray(xb[i * BLOC:(i + 1) * BLOC]),
            "w1t": w1t, "wct": wct, "gamma": gamma, "beta": beta,
        })
    res = run_bass_kernel_spmd(nc, in_maps, list(range(NCORES)))
    _CACHE["last_result"] = res
    out = np.concatenate([res.results[i]["out"] for i in range(NCORES)], axis=0)
    return out.astype(np.float32)


if __name__ == "__main__":
    rng = np.random.default_rng(0)
    inputs = {
        "x": rng.standard_normal((B, C, H, W)).astype(np.float32),
        "w_conv1": (rng.standard_normal((C, C, 4, 3)) * 0.02).astype(np.float32),
        "w_center": (rng.standard_normal((C, C, 1, 1)) * 0.05).astype(np.float32),
        "b_center": (rng.standard_normal((C,)) * 0.01).astype(np.float32),
        "gamma": np.ones(C, np.float32),
        "beta": np.zeros(C, np.float32),
    }
    out = kernel(**inputs)
    print("out", out.shape, out.dtype, float(np.abs(out).max()))
